# revision 3
# baseline (speedup 1.0000x reference)
"""Trainium2 Bass kernel for leave-one-out Nadaraya-Watson regression
(nn_Net_41420664602632, retrieval_knn).

Math
----
reference:
    Fx = x @ W.T ; Ft = train_X @ W.T          [N, 3]
    K[j,i,c] = exp(-((Ft[j,c]-Fx[i,c])/h)^2/2), K[i,i,c] = 0
    out[i,c] = sum_j K[j,i,c]*Y[j,c] / sum_j K[j,i,c]

With a = Ft/(sqrt(2)*h), b = Fx/(sqrt(2)*h) this is, per channel, a 1-D
Gaussian kernel regression: out[i] = numt(b_i)/dent(b_i) with
    numt(t) = sum_j Y_j exp(-(t-a_j)^2),  dent(t) = sum_j exp(-(t-a_j)^2)
numt/dent are Gaussian-smoothed fields with fixed width 1 in t-space
(the 1/(sqrt(2)h) scaling normalizes the bandwidth away), so instead of
evaluating them at all N=4096 query points (O(N^2) pairwise exps), the
device evaluates them on a uniform T=128-point grid covering the query
range (O(N*T)), and the host Catmull-Rom-interpolates at the 4096 query
positions (grid step ~0.03 of data range << kernel width => interp error
~1e-4 relative, far inside the 2e-2 gate; validated vs reference).

Device program (per core, j-shard of 512 training points)
---------------------------------------------------------
E[j,t] = exp(2 a_j g_t - a_j^2) via one ScalarE ACT per (c, j-tile):
    ACT(Exp, in=ramp[128,T] (grid bcast), scale=2a_j, bias=-a_j^2)
then PE accumulates num/den: [Y_j, 1]^T @ E -> PSUM[2, T] per channel
(channels on distinct PE col-groups + PSUM banks; jt-major issue order
so the 3 accumulation chains interleave on the PE).
Host sums the 8 cores' [2, 3T] partials, multiplies by exp(-g^2),
interpolates at b, subtracts the j==i self term, and divides.
"""

import numpy as np

import concourse.bass as bass
import concourse.tile as tile
from concourse import bacc, mybir
from concourse.bass_utils import run_bass_kernel_spmd

N = 4096       # training/query points
C = 3          # projected channels (fc1 out_features)
NCORES = 8
JSH = N // NCORES        # 512: j-shard per core
JTILES = JSH // 128      # 4
T = 128                  # grid targets (512B SBUF row -> line-rate DMA)

_CACHE = {}


def _build_nc(n=N, ncores=NCORES, t=T):
    key = (n, ncores, t)
    if key in _CACHE:
        return _CACHE[key]
    jtiles = (n // ncores) // 128
    f32 = mybir.dt.float32

    nc = bacc.Bacc("TRN2", target_bir_lowering=False, debug=False)
    # sb/st packed in one [128, 128] tensor (512B rows): scale/bias pairs
    # at cols 2k/2k+1, matmul weights (Y, 1) at cols 64+2k/64+2k+1
    sbst_d = nc.dram_tensor("sbst", [128, 128], f32, kind="ExternalInput")
    ramp_d = nc.dram_tensor("ramp", [1, t], f32, kind="ExternalInput")
    out_d = nc.dram_tensor("out", [2, C * t], f32, kind="ExternalOutput")

    with tile.TileContext(nc) as tc:
        with (
            tc.tile_pool(name="const", bufs=1) as constp,
            tc.tile_pool(name="g", bufs=4) as gp,
            tc.tile_pool(name="outsb", bufs=1) as outp,
            tc.tile_pool(name="psum", bufs=1, space=bass.MemorySpace.PSUM) as pp,
        ):
            # warm the ACT exp table immediately (no data dependencies)
            warm = constp.tile([128, 1], f32, tag="warm")
            zero_ap = nc.const_aps.scalar_like(0.0, warm[:])
            nc.scalar.activation(warm[:], zero_ap, mybir.ActivationFunctionType.Exp)

            # grid row -> all 128 partitions, on device (1 descriptor DMA
            # instead of a 64KB replicated load)
            ramp_row = constp.tile([1, t], f32, tag="ramp_row")
            ramp = constp.tile([128, t], f32, tag="ramp")
            nc.sync.dma_start(ramp_row[:], ramp_d.ap())
            nc.gpsimd.partition_broadcast(ramp[:], ramp_row[:])

            # sb/st: 64KB total, partition-chunks spread over the three
            # DMA-capable engines (sync / scalar-after-warm / gpsimd)
            sbst = constp.tile([128, 128], f32, tag="sbst")
            chunks = ((nc.sync, 0, 48), (nc.scalar, 48, 88), (nc.gpsimd, 88, 128))
            for eng, p0, p1 in chunks:
                eng.dma_start(sbst[p0:p1, :], sbst_d.ap()[p0:p1, :])

            acc = pp.tile([128, 4096], f32, tag="acc")
            outsb = outp.tile([2, C * t], f32, tag="outsb")

            # jt-major: the 3 channels' PSUM accumulation chains interleave
            # on distinct PE col-groups, so the PE never stalls on the
            # serial per-channel accumulate dependency
            for jt in range(jtiles):
                for c in range(C):
                    k = 2 * (c * jtiles + jt)
                    g = gp.tile([128, t], f32, tag="g")
                    nc.scalar.activation(
                        g[:],
                        ramp[:],
                        mybir.ActivationFunctionType.Exp,
                        bias=sbst[:, k + 1 : k + 2],
                        scale=sbst[:, k : k + 1],
                    )
                    nc.tensor.matmul(
                        acc[32 * c : 32 * c + 2, c * 512 : c * 512 + t],
                        lhsT=sbst[:, 64 + k : 64 + k + 2],
                        rhs=g[:],
                        start=(jt == 0),
                        stop=(jt == jtiles - 1),
                        tile_position=(0, 32 * c),
                    )
            for c in range(C):
                nc.vector.tensor_copy(
                    outsb[:, c * t : (c + 1) * t],
                    acc[32 * c : 32 * c + 2, c * 512 : c * 512 + t],
                )
            nc.sync.dma_start(out_d.ap(), outsb[:])

    nc.compile()
    _CACHE[key] = nc
    return nc


def _prep_inputs(x, train_X, Y, W, h, n=N, ncores=NCORES, t=T):
    """Host-side prep: projections, grid, per-core scale/bias maps."""
    jsh = n // ncores
    jtiles = jsh // 128
    x64 = np.asarray(x, np.float64)
    t64 = np.asarray(train_X, np.float64)
    W64 = np.asarray(W, np.float64)
    hv = float(np.asarray(h).reshape(-1)[0])
    s = 1.0 / (np.sqrt(2.0) * hv)
    b = (x64 @ W64.T) * s          # queries   [n, C]
    a = (t64 @ W64.T) * s          # training  [n, C]
    a32 = a.astype(np.float32)
    b32 = b.astype(np.float32)

    # uniform grid over the query range with a 2-step margin so every
    # query interpolates from an interior Catmull-Rom stencil
    minv = float(b32.min())
    maxv = float(b32.max())
    dg = (maxv - minv) / (t - 5) if maxv > minv else 1.0
    lo = minv - 2.0 * dg
    grid = (lo + dg * np.arange(t, dtype=np.float64)).astype(np.float32)

    ramp = grid.reshape(1, t)
    Yf = np.asarray(Y, np.float64).astype(np.float32)

    in_maps = []
    for r in range(ncores):
        j0 = r * jsh
        m = np.zeros((128, 128), np.float32)
        for c in range(C):
            for jt in range(jtiles):
                k = 2 * (c * jtiles + jt)
                aj = a32[j0 + jt * 128 : j0 + (jt + 1) * 128, c]
                m[:, k] = 2.0 * aj
                m[:, k + 1] = -(aj.astype(np.float64) ** 2).astype(np.float32)
                m[:, 64 + k] = Yf[j0 + jt * 128 : j0 + (jt + 1) * 128, c]
                m[:, 64 + k + 1] = 1.0
        in_maps.append({"sbst": m, "ramp": ramp})
    return in_maps, a32, b32, grid, lo, dg


def _interp_catmull_rom(f, lo, dg, xq, t=T):
    """Cubic Catmull-Rom interpolation of f (uniform grid) at xq."""
    u = (np.asarray(xq, np.float64) - lo) / dg
    i = np.clip(np.floor(u).astype(np.int64), 1, t - 3)
    u = u - i
    fm1, f0, f1, f2 = f[i - 1], f[i], f[i + 1], f[i + 2]
    return 0.5 * (
        2.0 * f0
        + (f1 - fm1) * u
        + (2.0 * fm1 - 5.0 * f0 + 4.0 * f1 - f2) * u * u
        + (3.0 * f0 - fm1 - 3.0 * f1 + f2) * u * u * u
    )


def _combine(results, Y, a32, b32, grid, lo, dg, n=N, t=T):
    """Sum per-core partials, damp, interpolate, self-subtract, divide."""
    num = np.zeros((C, t), np.float64)
    den = np.zeros((C, t), np.float64)
    for res in results:
        o = np.asarray(res["out"], np.float64)  # [2, C*t]
        num += o[0].reshape(C, t)
        den += o[1].reshape(C, t)
    damp = np.exp(-grid.astype(np.float64) ** 2)
    num *= damp
    den *= damp

    Yf = np.asarray(Y, np.float64)
    out = np.empty((n, C), np.float64)
    for c in range(C):
        ni = _interp_catmull_rom(num[c], lo, dg, b32[:, c], t)
        di = _interp_catmull_rom(den[c], lo, dg, b32[:, c], t)
        # leave-one-out: remove the j == i term exp(-(b_i - a_i)^2)
        kii = np.exp(
            -((b32[:, c].astype(np.float64) - a32[:, c].astype(np.float64)) ** 2)
        )
        out[:, c] = (ni - kii * Yf[:, c]) / (di - kii)
    return out.astype(np.float32)


def kernel(x, train_X, Y, W, h):
    nc = _build_nc()
    in_maps, a32, b32, grid, lo, dg = _prep_inputs(x, train_X, Y, W, h)
    res = run_bass_kernel_spmd(nc, in_maps, core_ids=list(range(NCORES)))
    return _combine(res.results, Y, a32, b32, grid, lo, dg)


# revision 5
# speedup vs baseline: 1.1676x; 1.1676x over previous
"""Trainium2 Bass kernel for leave-one-out Nadaraya-Watson regression
(nn_Net_41420664602632, retrieval_knn).

Math
----
reference:
    Fx = x @ W.T ; Ft = train_X @ W.T          [N, 3]
    K[j,i,c] = exp(-((Ft[j,c]-Fx[i,c])/h)^2/2), K[i,i,c] = 0
    out[i,c] = sum_j K[j,i,c]*Y[j,c] / sum_j K[j,i,c]

With a = Ft/(sqrt(2)*h), b = Fx/(sqrt(2)*h) this is, per channel, a 1-D
Gaussian kernel regression: out[i] = numt(b_i)/dent(b_i) with
    numt(t) = sum_j Y_j exp(-(t-a_j)^2),  dent(t) = sum_j exp(-(t-a_j)^2)
numt/dent are Gaussian-smoothed fields with fixed width 1 in t-space
(the 1/(sqrt(2)h) scaling normalizes the bandwidth away), so instead of
evaluating them at all N=4096 query points (O(N^2) pairwise exps), the
device evaluates them on a uniform T=128-point grid covering the query
range (O(N*T)), and the host Catmull-Rom-interpolates at the 4096 query
positions (grid step ~0.03 of data range << kernel width => interp error
~1e-4 relative, far inside the 2e-2 gate; validated vs reference).

Device program (per core, j-shard of 512 training points)
---------------------------------------------------------
The grid is an fp32 iota 0..T-1 (no DMA), and the affine grid transform
is folded into per-partition scalars computed on the host:
    arg[j,t] = (2 a_j dg) * t + (2 a_j lo - a_j^2)
Per j-tile: 3 DVE tensor_scalar ops build arg[128, 3T] (one slice per
channel), one ScalarE ACT takes exp of the whole [128, 3T] tile into
fp16, and 3 single-pass fp16 PE matmuls [Y_j, 1]^T @ E accumulate
num/den per channel into distinct PSUM banks / PE col-groups (jt-major
issue so the 3 accumulation chains interleave on the PE).
Host sums the 8 cores' [2, 3T] partials, multiplies by exp(-g^2),
interpolates at b, subtracts the j==i self term, and divides.
"""

import numpy as np

import concourse.bass as bass
import concourse.tile as tile
from concourse import bacc, mybir
from concourse.bass_utils import run_bass_kernel_spmd

N = 4096       # training/query points
C = 3          # projected channels (fc1 out_features)
NCORES = 8
JSH = N // NCORES        # 512: j-shard per core
JTILES = JSH // 128      # 4
T = 128                  # grid targets

_CACHE = {}


def _build_nc(n=N, ncores=NCORES, t=T):
    key = (n, ncores, t)
    if key in _CACHE:
        return _CACHE[key]
    jtiles = (n // ncores) // 128
    f32 = mybir.dt.float32
    f16 = mybir.dt.float16

    nc = bacc.Bacc("TRN2", target_bir_lowering=False, debug=False)
    # one [128, 128] f32 input (512B rows -> line-rate DMA):
    #   cols 2m/2m+1 (m = c*jtiles+jt): DVE scale' = 2*a*dg, bias' = 2*a*lo-a^2
    #   col 64+m: (Y[j,c], 1.0) packed as two fp16 -> matmul lhsT via bitcast
    sbst_d = nc.dram_tensor("sbst", [128, 128], f32, kind="ExternalInput")
    out_d = nc.dram_tensor("out", [2, C * t], f32, kind="ExternalOutput")

    with tile.TileContext(nc) as tc:
        with (
            tc.tile_pool(name="const", bufs=1) as constp,
            tc.tile_pool(name="arg", bufs=2) as argp,
            tc.tile_pool(name="g", bufs=2) as gp,
            tc.tile_pool(name="outsb", bufs=1) as outp,
            tc.tile_pool(name="psum", bufs=1, space=bass.MemorySpace.PSUM) as pp,
        ):
            # warm the ACT exp table immediately (no data dependencies)
            warm = constp.tile([128, 1], f32, tag="warm")
            zero_ap = nc.const_aps.scalar_like(0.0, warm[:])
            nc.scalar.activation(warm[:], zero_ap, mybir.ActivationFunctionType.Exp)

            # grid 0..t-1 generated on device (exact in fp32 for t<=2^24)
            ramp = constp.tile([128, t], f32, tag="ramp")
            nc.gpsimd.iota(
                ramp[:],
                [[1, t]],
                channel_multiplier=0,
                allow_small_or_imprecise_dtypes=True,
            )

            # 64KB input, partition-chunks on the three DMA-capable engines
            sbst = constp.tile([128, 128], f32, tag="sbst")
            chunks = ((nc.sync, 0, 48), (nc.scalar, 48, 88), (nc.gpsimd, 88, 128))
            for eng, p0, p1 in chunks:
                eng.dma_start(sbst[p0:p1, :], sbst_d.ap()[p0:p1, :])

            acc = pp.tile([128, 4096], f32, tag="acc")
            outsb = outp.tile([2, C * t], f32, tag="outsb")

            for jt in range(jtiles):
                arg = argp.tile([128, C * t], f32, tag="arg")
                for c in range(C):
                    k = 2 * (c * jtiles + jt)
                    nc.vector.tensor_scalar(
                        arg[:, c * t : (c + 1) * t],
                        ramp[:],
                        sbst[:, k : k + 1],
                        sbst[:, k + 1 : k + 2],
                        mybir.AluOpType.mult,
                        mybir.AluOpType.add,
                    )
                g = gp.tile([128, C * t], f16, tag="g")
                nc.scalar.activation(
                    g[:], arg[:], mybir.ActivationFunctionType.Exp
                )
                for c in range(C):
                    m = c * jtiles + jt
                    nc.tensor.matmul(
                        acc[32 * c : 32 * c + 2, c * 512 : c * 512 + t],
                        lhsT=sbst[:, 64 + m : 64 + m + 1].bitcast(f16),
                        rhs=g[:, c * t : (c + 1) * t],
                        start=(jt == 0),
                        stop=(jt == jtiles - 1),
                        tile_position=(0, 32 * c),
                    )
            for c in range(C):
                nc.vector.tensor_copy(
                    outsb[:, c * t : (c + 1) * t],
                    acc[32 * c : 32 * c + 2, c * 512 : c * 512 + t],
                )
            nc.sync.dma_start(out_d.ap(), outsb[:])

    nc.compile()
    _CACHE[key] = nc
    return nc


def _prep_inputs(x, train_X, Y, W, h, n=N, ncores=NCORES, t=T):
    """Host-side prep: projections, grid, per-core scale/bias maps."""
    jsh = n // ncores
    jtiles = jsh // 128
    x64 = np.asarray(x, np.float64)
    t64 = np.asarray(train_X, np.float64)
    W64 = np.asarray(W, np.float64)
    hv = float(np.asarray(h).reshape(-1)[0])
    s = 1.0 / (np.sqrt(2.0) * hv)
    b = (x64 @ W64.T) * s          # queries   [n, C]
    a = (t64 @ W64.T) * s          # training  [n, C]
    a32 = a.astype(np.float32)
    b32 = b.astype(np.float32)

    # uniform grid over the query range with a 2-step margin so every
    # query interpolates from an interior Catmull-Rom stencil
    minv = float(b32.min())
    maxv = float(b32.max())
    dg = (maxv - minv) / (t - 5) if maxv > minv else 1.0
    lo = minv - 2.0 * dg

    Yf = np.asarray(Y, np.float64).astype(np.float32)
    one16 = np.uint32(np.float16(1.0).view(np.uint16)) << np.uint32(16)

    in_maps = []
    for r in range(ncores):
        j0 = r * jsh
        m = np.zeros((128, 128), np.float32)
        mu = m.view(np.uint32)
        for c in range(C):
            for jt in range(jtiles):
                kk = c * jtiles + jt
                aj = a32[j0 + jt * 128 : j0 + (jt + 1) * 128, c].astype(np.float64)
                m[:, 2 * kk] = (2.0 * aj * dg).astype(np.float32)
                m[:, 2 * kk + 1] = (2.0 * aj * lo - aj * aj).astype(np.float32)
                y16 = Yf[j0 + jt * 128 : j0 + (jt + 1) * 128, c].astype(np.float16)
                mu[:, 64 + kk] = y16.view(np.uint16).astype(np.uint32) | one16
        in_maps.append({"sbst": m})
    return in_maps, a32, b32, lo, dg


def _interp_catmull_rom(f, lo, dg, xq, t=T):
    """Cubic Catmull-Rom interpolation of f (uniform grid) at xq."""
    u = (np.asarray(xq, np.float64) - lo) / dg
    i = np.clip(np.floor(u).astype(np.int64), 1, t - 3)
    u = u - i
    fm1, f0, f1, f2 = f[i - 1], f[i], f[i + 1], f[i + 2]
    return 0.5 * (
        2.0 * f0
        + (f1 - fm1) * u
        + (2.0 * fm1 - 5.0 * f0 + 4.0 * f1 - f2) * u * u
        + (3.0 * f0 - fm1 - 3.0 * f1 + f2) * u * u * u
    )


def _combine(results, Y, a32, b32, lo, dg, n=N, t=T):
    """Sum per-core partials, damp, interpolate, self-subtract, divide."""
    num = np.zeros((C, t), np.float64)
    den = np.zeros((C, t), np.float64)
    for res in results:
        o = np.asarray(res["out"], np.float64)  # [2, C*t]
        num += o[0].reshape(C, t)
        den += o[1].reshape(C, t)
    grid = lo + dg * np.arange(t, dtype=np.float64)
    damp = np.exp(-(grid**2))
    num *= damp
    den *= damp

    Yf = np.asarray(Y, np.float64)
    out = np.empty((n, C), np.float64)
    for c in range(C):
        ni = _interp_catmull_rom(num[c], lo, dg, b32[:, c], t)
        di = _interp_catmull_rom(den[c], lo, dg, b32[:, c], t)
        # leave-one-out: remove the j == i term exp(-(b_i - a_i)^2)
        kii = np.exp(
            -((b32[:, c].astype(np.float64) - a32[:, c].astype(np.float64)) ** 2)
        )
        out[:, c] = (ni - kii * Yf[:, c]) / (di - kii)
    return out.astype(np.float32)


def kernel(x, train_X, Y, W, h):
    nc = _build_nc()
    in_maps, a32, b32, lo, dg = _prep_inputs(x, train_X, Y, W, h)
    res = run_bass_kernel_spmd(nc, in_maps, core_ids=list(range(NCORES)))
    return _combine(res.results, Y, a32, b32, lo, dg)


# revision 10
# speedup vs baseline: 1.3681x; 1.1717x over previous
"""Trainium2 Bass kernel for leave-one-out Nadaraya-Watson regression
(nn_Net_41420664602632, retrieval_knn).

Math
----
reference:
    Fx = x @ W.T ; Ft = train_X @ W.T          [N, 3]
    K[j,i,c] = exp(-((Ft[j,c]-Fx[i,c])/h)^2/2), K[i,i,c] = 0
    out[i,c] = sum_j K[j,i,c]*Y[j,c] / sum_j K[j,i,c]

With a = Ft/(sqrt(2)*h), b = Fx/(sqrt(2)*h) this is, per channel, a 1-D
Gaussian kernel regression: out[i] = numt(b_i)/dent(b_i) with
    numt(t) = sum_j Y_j exp(-(t-a_j)^2),  dent(t) = sum_j exp(-(t-a_j)^2)
numt/dent are Gaussian-smoothed fields with fixed width 1 in t-space
(the 1/(sqrt(2)h) scaling normalizes the bandwidth away), so instead of
evaluating them at all N=4096 query points (O(N^2) pairwise exps), the
device evaluates them on a uniform T=128-point grid covering the query
range (O(N*T)), and the host Catmull-Rom-interpolates at the 4096 query
positions (grid step ~0.03 of data range << kernel width => interp error
~1e-4 relative, far inside the 2e-2 gate; validated vs reference).

Device program (per core, j-shard of 512 training points)
---------------------------------------------------------
The grid is an fp32 iota 0..T-1 (no DMA), and the affine grid transform
is folded into per-partition scalars computed on the host:
    arg[j,t] = (2 a_j dg) * t + (2 a_j lo - a_j^2)
Per j-tile: 3 DVE tensor_scalar ops build arg[128, 3T] (one slice per
channel), one ScalarE ACT takes exp of the whole [128, 3T] tile into
fp16, and 3 single-pass fp16 PE matmuls [Y_j, 1]^T @ E accumulate
num/den per channel into distinct PSUM banks / PE col-groups (jt-major
issue so the 3 accumulation chains interleave on the PE).
Host sums the 8 cores' [2, 3T] partials, multiplies by exp(-g^2),
interpolates at b, subtracts the j==i self term, and divides.
"""

import numpy as np

import concourse.bass as bass
import concourse.tile as tile
from concourse import bacc, mybir
from concourse.bass_utils import run_bass_kernel_spmd

N = 4096       # training/query points
C = 3          # projected channels (fc1 out_features)
NCORES = 8
JSH = N // NCORES        # 512: j-shard per core
JTILES = JSH // 128      # 4
T = 128                  # grid targets

_CACHE = {}


def _build_nc(n=N, ncores=NCORES, t=T):
    key = (n, ncores, t)
    if key in _CACHE:
        return _CACHE[key]
    jtiles = (n // ncores) // 128
    f32 = mybir.dt.float32
    f16 = mybir.dt.float16

    nc = bacc.Bacc("TRN2", target_bir_lowering=False, debug=False)
    # one [128, 128] f32 input (512B rows -> line-rate DMA):
    #   cols 2m/2m+1 (m = c*jtiles+jt): DVE scale' = 2*a*dg, bias' = 2*a*lo-a^2
    #   col 64+m: (Y[j,c], 1.0) packed as two fp16 -> matmul lhsT via bitcast
    sbst_d = nc.dram_tensor("sbst", [128, 128], f32, kind="ExternalInput")
    out_d = nc.dram_tensor("out", [2, C * t], f32, kind="ExternalOutput")

    with tile.TileContext(nc) as tc:
        with (
            tc.tile_pool(name="const", bufs=1) as constp,
            tc.tile_pool(name="arg", bufs=2) as argp,
            tc.tile_pool(name="g", bufs=2) as gp,
            tc.tile_pool(name="outsb", bufs=1) as outp,
            tc.tile_pool(name="psum", bufs=1, space=bass.MemorySpace.PSUM) as pp,
        ):
            # 64KB input, partition-chunks on the three DMA-capable engines.
            # These issue first: the whole compute stream waits on them.
            sbst = constp.tile([128, 128], f32, tag="sbst")
            chunks = ((nc.sync, 0, 48), (nc.scalar, 48, 88), (nc.gpsimd, 88, 128))
            for eng, p0, p1 in chunks:
                eng.dma_start(sbst[p0:p1, :], sbst_d.ap()[p0:p1, :])

            # warm the ACT exp table (after the scalar-queue DMA issue so it
            # doesn't delay the chunk; its table load still beats the 1st ACT)
            warm = constp.tile([128, 1], f32, tag="warm")
            zero_ap = nc.const_aps.scalar_like(0.0, warm[:])
            nc.scalar.activation(warm[:], zero_ap, mybir.ActivationFunctionType.Exp)

            # grid 0..t-1 generated on device (exact in fp32 for t<=2^24)
            ramp = constp.tile([128, t], f32, tag="ramp")
            nc.gpsimd.iota(
                ramp[:],
                [[1, t]],
                channel_multiplier=0,
                allow_small_or_imprecise_dtypes=True,
            )

            acc = pp.tile([128, 4096], f32, tag="acc")
            outsb = outp.tile([2, C * t], f32, tag="outsb")

            for jt in range(jtiles):
                arg = argp.tile([128, C * t], f32, tag="arg")
                for c in range(C):
                    k = 2 * (c * jtiles + jt)
                    # alternate DVE/GpSimd so the two elementwise engines
                    # split the 12 affine ops and neither paces the ACTs
                    eng = nc.vector if (jt * C + c) % 2 == 0 else nc.gpsimd
                    eng.tensor_scalar(
                        arg[:, c * t : (c + 1) * t],
                        ramp[:],
                        sbst[:, k : k + 1],
                        sbst[:, k + 1 : k + 2],
                        mybir.AluOpType.mult,
                        mybir.AluOpType.add,
                    )
                g = gp.tile([128, C * t], f16, tag="g")
                nc.scalar.activation(
                    g[:], arg[:], mybir.ActivationFunctionType.Exp
                )
                for c in range(C):
                    m = c * jtiles + jt
                    nc.tensor.matmul(
                        acc[32 * c : 32 * c + 2, c * 512 : c * 512 + t],
                        lhsT=sbst[:, 64 + m : 64 + m + 1].bitcast(f16),
                        rhs=g[:, c * t : (c + 1) * t],
                        start=(jt == 0),
                        stop=(jt == jtiles - 1),
                        tile_position=(0, 32 * c),
                    )
            # evacuate each channel on a different engine as its
            # accumulation closes, then one small DMA out
            for c, eng in ((0, nc.vector), (1, nc.scalar), (2, nc.vector)):
                src = acc[32 * c : 32 * c + 2, c * 512 : c * 512 + t]
                dst = outsb[:, c * t : (c + 1) * t]
                if eng is nc.scalar:
                    eng.copy(dst, src)
                else:
                    eng.tensor_copy(dst, src)
            nc.sync.dma_start(out_d.ap(), outsb[:])

    nc.compile()
    _CACHE[key] = nc
    return nc


def _prep_inputs(x, train_X, Y, W, h, n=N, ncores=NCORES, t=T):
    """Host-side prep: projections, grid, per-core scale/bias maps."""
    jsh = n // ncores
    jtiles = jsh // 128
    x64 = np.asarray(x, np.float64)
    t64 = np.asarray(train_X, np.float64)
    W64 = np.asarray(W, np.float64)
    hv = float(np.asarray(h).reshape(-1)[0])
    s = 1.0 / (np.sqrt(2.0) * hv)
    b = (x64 @ W64.T) * s          # queries   [n, C]
    a = (t64 @ W64.T) * s          # training  [n, C]
    a32 = a.astype(np.float32)
    b32 = b.astype(np.float32)

    # uniform grid over the query range with a 2-step margin so every
    # query interpolates from an interior Catmull-Rom stencil
    minv = float(b32.min())
    maxv = float(b32.max())
    dg = (maxv - minv) / (t - 5) if maxv > minv else 1.0
    lo = minv - 2.0 * dg

    Yf = np.asarray(Y, np.float64).astype(np.float32)
    one16 = np.uint32(np.float16(1.0).view(np.uint16)) << np.uint32(16)

    in_maps = []
    for r in range(ncores):
        j0 = r * jsh
        m = np.zeros((128, 128), np.float32)
        mu = m.view(np.uint32)
        for c in range(C):
            for jt in range(jtiles):
                kk = c * jtiles + jt
                aj = a32[j0 + jt * 128 : j0 + (jt + 1) * 128, c].astype(np.float64)
                m[:, 2 * kk] = (2.0 * aj * dg).astype(np.float32)
                m[:, 2 * kk + 1] = (2.0 * aj * lo - aj * aj).astype(np.float32)
                y16 = Yf[j0 + jt * 128 : j0 + (jt + 1) * 128, c].astype(np.float16)
                mu[:, 64 + kk] = y16.view(np.uint16).astype(np.uint32) | one16
        in_maps.append({"sbst": m})
    return in_maps, a32, b32, lo, dg


def _interp_catmull_rom(f, lo, dg, xq, t=T):
    """Cubic Catmull-Rom interpolation of f (uniform grid) at xq."""
    u = (np.asarray(xq, np.float64) - lo) / dg
    i = np.clip(np.floor(u).astype(np.int64), 1, t - 3)
    u = u - i
    fm1, f0, f1, f2 = f[i - 1], f[i], f[i + 1], f[i + 2]
    return 0.5 * (
        2.0 * f0
        + (f1 - fm1) * u
        + (2.0 * fm1 - 5.0 * f0 + 4.0 * f1 - f2) * u * u
        + (3.0 * f0 - fm1 - 3.0 * f1 + f2) * u * u * u
    )


def _combine(results, Y, a32, b32, lo, dg, n=N, t=T):
    """Sum per-core partials, damp, interpolate, self-subtract, divide."""
    num = np.zeros((C, t), np.float64)
    den = np.zeros((C, t), np.float64)
    for res in results:
        o = np.asarray(res["out"], np.float64)  # [2, C*t]
        num += o[0].reshape(C, t)
        den += o[1].reshape(C, t)
    grid = lo + dg * np.arange(t, dtype=np.float64)
    damp = np.exp(-(grid**2))
    num *= damp
    den *= damp

    Yf = np.asarray(Y, np.float64)
    out = np.empty((n, C), np.float64)
    for c in range(C):
        ni = _interp_catmull_rom(num[c], lo, dg, b32[:, c], t)
        di = _interp_catmull_rom(den[c], lo, dg, b32[:, c], t)
        # leave-one-out: remove the j == i term exp(-(b_i - a_i)^2)
        kii = np.exp(
            -((b32[:, c].astype(np.float64) - a32[:, c].astype(np.float64)) ** 2)
        )
        out[:, c] = (ni - kii * Yf[:, c]) / (di - kii)
    return out.astype(np.float32)


def kernel(x, train_X, Y, W, h):
    nc = _build_nc()
    in_maps, a32, b32, lo, dg = _prep_inputs(x, train_X, Y, W, h)
    res = run_bass_kernel_spmd(nc, in_maps, core_ids=list(range(NCORES)))
    return _combine(res.results, Y, a32, b32, lo, dg)


# revision 12
# speedup vs baseline: 1.4984x; 1.0952x over previous
"""Trainium2 Bass kernel for leave-one-out Nadaraya-Watson regression
(nn_Net_41420664602632, retrieval_knn).

Math
----
reference:
    Fx = x @ W.T ; Ft = train_X @ W.T          [N, 3]
    K[j,i,c] = exp(-((Ft[j,c]-Fx[i,c])/h)^2/2), K[i,i,c] = 0
    out[i,c] = sum_j K[j,i,c]*Y[j,c] / sum_j K[j,i,c]

With a = Ft/(sqrt(2)*h), b = Fx/(sqrt(2)*h) this is, per channel, a 1-D
Gaussian kernel regression: out[i] = numt(b_i)/dent(b_i) with
    numt(t) = sum_j Y_j exp(-(t-a_j)^2),  dent(t) = sum_j exp(-(t-a_j)^2)
numt/dent are Gaussian-smoothed fields with fixed width 1 in t-space
(the 1/(sqrt(2)h) scaling normalizes the bandwidth away), so instead of
evaluating them at all N=4096 query points (O(N^2) pairwise exps), the
device evaluates them on a uniform T=128-point grid covering the query
range (O(N*T)), and the host Catmull-Rom-interpolates at the 4096 query
positions (grid step ~0.03 of data range << kernel width => interp error
~3e-4 relative, far inside the 2e-2 gate; validated vs reference).

Device program (per core, j-shard of 512 training points)
---------------------------------------------------------
Hand-scheduled Bass (no TileContext): per-engine instruction queues with
manual semaphores. The TileContext scheduler adds ~0.3-0.7us of
semaphore bookkeeping around every instruction plus a ~5us exit sweep
that resets every allocated semaphore on every engine; with only ~50
real instructions this overhead dominated, so the program is wired by
hand (no buffer reuse -> no WAR hazards, 7 semaphores total).

The grid is an fp32 iota 0..T-1 (no DMA), and the affine grid transform
is folded into per-partition scalars computed on the host:
    arg[j,t] = (2 a_j dg) * t + (2 a_j lo - a_j^2)
Per j-tile: 3 tensor_scalar ops (split across DVE and GpSimd) build
arg[128, 3T], one ScalarE ACT takes exp of the whole tile into fp16,
and 3 single-pass fp16 PE matmuls [Y_j, 1]^T @ E accumulate num/den per
channel into distinct PSUM banks / PE col-groups.
Host sums the 8 cores' [2, 3T] partials, multiplies by exp(-g^2),
interpolates at b, subtracts the j==i self term, and divides.
"""

import numpy as np

import concourse.bass as bass
from concourse import bacc, mybir
from concourse.bass_utils import run_bass_kernel_spmd

N = 4096       # training/query points
C = 3          # projected channels (fc1 out_features)
NCORES = 8
JSH = N // NCORES        # 512: j-shard per core
JTILES = JSH // 128      # 4
T = 128                  # grid targets

_CACHE = {}


def _build_nc(n=N, ncores=NCORES, t=T):
    key = (n, ncores, t)
    if key in _CACHE:
        return _CACHE[key]
    jtiles = (n // ncores) // 128
    f32 = mybir.dt.float32
    f16 = mybir.dt.float16
    Exp = mybir.ActivationFunctionType.Exp

    nc = bacc.Bacc("TRN2", target_bir_lowering=False, debug=False)
    # one [128, 128] f32 input (512B rows -> line-rate DMA):
    #   cols 2m/2m+1 (m = c*jtiles+jt): scale' = 2*a*dg, bias' = 2*a*lo-a^2
    #   col 64+m: (Y[j,c], 1.0) packed as two fp16 -> matmul lhsT via bitcast
    sbst_d = nc.dram_tensor("sbst", [128, 128], f32, kind="ExternalInput")
    out_d = nc.dram_tensor("out", [2, C * t], f32, kind="ExternalOutput")

    sbst = nc.alloc_sbuf_tensor("sbst_sb", [128, 128], f32)
    ramp = nc.alloc_sbuf_tensor("ramp_sb", [128, t], f32)
    args = nc.alloc_sbuf_tensor("args_sb", [128, jtiles * C * t], f32)
    gbuf = nc.alloc_sbuf_tensor("g_sb", [128, jtiles * C * t], f16)
    outsb = nc.alloc_sbuf_tensor("out_sb", [2, C * t], f32)
    acc = nc.alloc_psum_tensor("acc_ps", [128, 2048], f32)

    s_in = nc.alloc_semaphore("s_in")      # input DMA chunks (+16 each)
    s_ramp = nc.alloc_semaphore("s_ramp")  # iota done
    s_argv = nc.alloc_semaphore("s_argv")  # DVE arg ops done
    s_argg = nc.alloc_semaphore("s_argg")  # GpSimd arg ops done
    s_g = nc.alloc_semaphore("s_g")        # exp tiles done
    s_mm = nc.alloc_semaphore("s_mm")      # per-channel accumulation closed
    s_ev = nc.alloc_semaphore("s_ev")      # evacuation copies done
    s_out = nc.alloc_semaphore("s_out")    # output DMA done

    # which engine computes arg (jt, c): alternate DVE / GpSimd
    def arg_eng(jt, c):
        return nc.vector if (jt * C + c) % 2 == 0 else nc.gpsimd

    # cumulative arg-op counts per engine after each jt batch
    nv = [0] * jtiles
    ng = [0] * jtiles
    v = g = 0
    for jt in range(jtiles):
        for c in range(C):
            if arg_eng(jt, c) is nc.vector:
                v += 1
            else:
                g += 1
        nv[jt], ng[jt] = v, g

    aslc = lambda jt, c: args.ap()[:, (jt * C + c) * t : (jt * C + c + 1) * t]
    gslc = lambda jt, c: gbuf.ap()[:, (jt * C + c) * t : (jt * C + c + 1) * t]

    # --- sync: input chunk 0, then final output DMA ---
    nc.sync.dma_start(sbst.ap()[0:48, :], sbst_d.ap()[0:48, :]).then_inc(s_in, 16)
    nc.sync.wait_ge(s_ev, C)
    nc.sync.dma_start(out_d.ap(), outsb.ap()).then_inc(s_out, 16)
    nc.sync.wait_ge(s_out, 16)

    # --- scalar: input chunk 1, exp-table warm, the 4 exp ACTs, evac c1 ---
    nc.scalar.dma_start(sbst.ap()[48:88, :], sbst_d.ap()[48:88, :]).then_inc(
        s_in, 16
    )
    warm = nc.alloc_sbuf_tensor("warm_sb", [128, 1], f32)
    nc.scalar.activation(warm.ap(), nc.const_aps.scalar_like(0.0, warm.ap()), Exp)
    for jt in range(jtiles):
        nc.scalar.wait_ge(s_argv, nv[jt])
        nc.scalar.wait_ge(s_argg, ng[jt])
        nc.scalar.activation(
            gbuf.ap()[:, jt * C * t : (jt + 1) * C * t],
            args.ap()[:, jt * C * t : (jt + 1) * C * t],
            Exp,
        ).then_inc(s_g)
    nc.scalar.wait_ge(s_mm, 2)
    nc.scalar.copy(
        outsb.ap()[:, t : 2 * t], acc.ap()[32 : 32 + 2, 512 : 512 + t]
    ).then_inc(s_ev)

    # --- vector: 6 arg ops, evac c0 and c2 ---
    nc.vector.wait_ge(s_in, 48)
    nc.vector.wait_ge(s_ramp, 1)
    for jt in range(jtiles):
        for c in range(C):
            if arg_eng(jt, c) is nc.vector:
                k = 2 * (c * jtiles + jt)
                nc.vector.tensor_scalar(
                    aslc(jt, c),
                    ramp.ap(),
                    sbst.ap()[:, k : k + 1],
                    sbst.ap()[:, k + 1 : k + 2],
                    mybir.AluOpType.mult,
                    mybir.AluOpType.add,
                ).then_inc(s_argv)
    nc.vector.wait_ge(s_mm, 1)
    nc.vector.tensor_copy(
        outsb.ap()[:, 0:t], acc.ap()[0:2, 0:t]
    ).then_inc(s_ev)
    nc.vector.wait_ge(s_mm, 3)
    nc.vector.tensor_copy(
        outsb.ap()[:, 2 * t : 3 * t], acc.ap()[64 : 64 + 2, 1024 : 1024 + t]
    ).then_inc(s_ev)

    # --- gpsimd: input chunk 2, iota ramp, 6 arg ops ---
    nc.gpsimd.dma_start(sbst.ap()[88:128, :], sbst_d.ap()[88:128, :]).then_inc(
        s_in, 16
    )
    nc.gpsimd.iota(
        ramp.ap(), [[1, t]], channel_multiplier=0,
        allow_small_or_imprecise_dtypes=True,
    ).then_inc(s_ramp)
    nc.gpsimd.wait_ge(s_in, 48)
    for jt in range(jtiles):
        for c in range(C):
            if arg_eng(jt, c) is nc.gpsimd:
                k = 2 * (c * jtiles + jt)
                nc.gpsimd.tensor_scalar(
                    aslc(jt, c),
                    ramp.ap(),
                    sbst.ap()[:, k : k + 1],
                    sbst.ap()[:, k + 1 : k + 2],
                    mybir.AluOpType.mult,
                    mybir.AluOpType.add,
                ).then_inc(s_argg)

    # --- tensor: 12 fp16 matmuls, 3 channels on distinct col-groups ---
    nc.tensor.wait_ge(s_in, 48)
    for jt in range(jtiles):
        nc.tensor.wait_ge(s_g, jt + 1)
        for c in range(C):
            m = c * jtiles + jt
            mm = nc.tensor.matmul(
                acc.ap()[32 * c : 32 * c + 2, c * 512 : c * 512 + t],
                lhsT=sbst.ap()[:, 64 + m : 64 + m + 1].bitcast(f16),
                rhs=gslc(jt, c),
                start=(jt == 0),
                stop=(jt == jtiles - 1),
                tile_position=(0, 32 * c),
            )
            if jt == jtiles - 1:
                mm.then_inc(s_mm)

    nc.compile()
    _CACHE[key] = nc
    return nc


def _prep_inputs(x, train_X, Y, W, h, n=N, ncores=NCORES, t=T):
    """Host-side prep: projections, grid, per-core scale/bias maps."""
    jsh = n // ncores
    jtiles = jsh // 128
    x64 = np.asarray(x, np.float64)
    t64 = np.asarray(train_X, np.float64)
    W64 = np.asarray(W, np.float64)
    hv = float(np.asarray(h).reshape(-1)[0])
    s = 1.0 / (np.sqrt(2.0) * hv)
    b = (x64 @ W64.T) * s          # queries   [n, C]
    a = (t64 @ W64.T) * s          # training  [n, C]
    a32 = a.astype(np.float32)
    b32 = b.astype(np.float32)

    # uniform grid over the query range with a 2-step margin so every
    # query interpolates from an interior Catmull-Rom stencil
    minv = float(b32.min())
    maxv = float(b32.max())
    dg = (maxv - minv) / (t - 5) if maxv > minv else 1.0
    lo = minv - 2.0 * dg

    Yf = np.asarray(Y, np.float64).astype(np.float32)
    one16 = np.uint32(np.float16(1.0).view(np.uint16)) << np.uint32(16)

    in_maps = []
    for r in range(ncores):
        j0 = r * jsh
        m = np.zeros((128, 128), np.float32)
        mu = m.view(np.uint32)
        for c in range(C):
            for jt in range(jtiles):
                kk = c * jtiles + jt
                aj = a32[j0 + jt * 128 : j0 + (jt + 1) * 128, c].astype(np.float64)
                m[:, 2 * kk] = (2.0 * aj * dg).astype(np.float32)
                m[:, 2 * kk + 1] = (2.0 * aj * lo - aj * aj).astype(np.float32)
                y16 = Yf[j0 + jt * 128 : j0 + (jt + 1) * 128, c].astype(np.float16)
                mu[:, 64 + kk] = y16.view(np.uint16).astype(np.uint32) | one16
        in_maps.append({"sbst": m})
    return in_maps, a32, b32, lo, dg


def _interp_catmull_rom(f, lo, dg, xq, t=T):
    """Cubic Catmull-Rom interpolation of f (uniform grid) at xq."""
    u = (np.asarray(xq, np.float64) - lo) / dg
    i = np.clip(np.floor(u).astype(np.int64), 1, t - 3)
    u = u - i
    fm1, f0, f1, f2 = f[i - 1], f[i], f[i + 1], f[i + 2]
    return 0.5 * (
        2.0 * f0
        + (f1 - fm1) * u
        + (2.0 * fm1 - 5.0 * f0 + 4.0 * f1 - f2) * u * u
        + (3.0 * f0 - fm1 - 3.0 * f1 + f2) * u * u * u
    )


def _combine(results, Y, a32, b32, lo, dg, n=N, t=T):
    """Sum per-core partials, damp, interpolate, self-subtract, divide."""
    num = np.zeros((C, t), np.float64)
    den = np.zeros((C, t), np.float64)
    for res in results:
        o = np.asarray(res["out"], np.float64)  # [2, C*t]
        num += o[0].reshape(C, t)
        den += o[1].reshape(C, t)
    grid = lo + dg * np.arange(t, dtype=np.float64)
    damp = np.exp(-(grid**2))
    num *= damp
    den *= damp

    Yf = np.asarray(Y, np.float64)
    out = np.empty((n, C), np.float64)
    for c in range(C):
        ni = _interp_catmull_rom(num[c], lo, dg, b32[:, c], t)
        di = _interp_catmull_rom(den[c], lo, dg, b32[:, c], t)
        # leave-one-out: remove the j == i term exp(-(b_i - a_i)^2)
        kii = np.exp(
            -((b32[:, c].astype(np.float64) - a32[:, c].astype(np.float64)) ** 2)
        )
        out[:, c] = (ni - kii * Yf[:, c]) / (di - kii)
    return out.astype(np.float32)


def kernel(x, train_X, Y, W, h):
    nc = _build_nc()
    in_maps, a32, b32, lo, dg = _prep_inputs(x, train_X, Y, W, h)
    res = run_bass_kernel_spmd(nc, in_maps, core_ids=list(range(NCORES)))
    return _combine(res.results, Y, a32, b32, lo, dg)


# revision 14
# speedup vs baseline: 1.7008x; 1.1351x over previous
"""Trainium2 Bass kernel for leave-one-out Nadaraya-Watson regression
(nn_Net_41420664602632, retrieval_knn).

Math
----
reference:
    Fx = x @ W.T ; Ft = train_X @ W.T          [N, 3]
    K[j,i,c] = exp(-((Ft[j,c]-Fx[i,c])/h)^2/2), K[i,i,c] = 0
    out[i,c] = sum_j K[j,i,c]*Y[j,c] / sum_j K[j,i,c]

With a = Ft/(sqrt(2)*h), b = Fx/(sqrt(2)*h) this is, per channel, a 1-D
Gaussian kernel regression: out[i] = numt(b_i)/dent(b_i) with
    numt(t) = sum_j Y_j exp(-(t-a_j)^2),  dent(t) = sum_j exp(-(t-a_j)^2)
numt/dent are Gaussian-smoothed fields with fixed width 1 in t-space
(the 1/(sqrt(2)h) scaling normalizes the bandwidth away), so instead of
evaluating them at all N=4096 query points (O(N^2) pairwise exps), the
device evaluates them on a uniform T=128-point grid covering the query
range (O(N*T)), and the host Catmull-Rom-interpolates at the 4096 query
positions (grid step ~0.03 of data range << kernel width => interp error
~3e-4 relative, far inside the 2e-2 gate; validated vs reference).

Device program (per core, j-shard of 512 training points)
---------------------------------------------------------
Hand-scheduled Bass (no TileContext): per-engine instruction queues with
manual semaphores. The TileContext scheduler adds ~0.3-0.7us of
semaphore bookkeeping around every instruction plus a ~5us exit sweep
that resets every allocated semaphore on every engine; with only ~50
real instructions this overhead dominated, so the program is wired by
hand (no buffer reuse -> no WAR hazards, 7 semaphores total).

The grid is an fp32 iota 0..T-1 (no DMA), and the affine grid transform
is folded into per-partition scalars computed on the host:
    arg[j,t] = (2 a_j dg) * t + (2 a_j lo - a_j^2)
Per j-tile: 3 tensor_scalar ops (split across DVE and GpSimd) build
arg[128, 3T], one ScalarE ACT takes exp of the whole tile into fp16,
and 3 single-pass fp16 PE matmuls [Y_j, 1]^T @ E accumulate num/den per
channel into distinct PSUM banks / PE col-groups.
Host sums the 8 cores' [2, 3T] partials, multiplies by exp(-g^2),
interpolates at b, subtracts the j==i self term, and divides.
"""

import numpy as np

import concourse.bass as bass
from concourse import bacc, mybir
from concourse.bass_utils import run_bass_kernel_spmd

N = 4096       # training/query points
C = 3          # projected channels (fc1 out_features)
NCORES = 8
JSH = N // NCORES        # 512: j-shard per core
JTILES = JSH // 128      # 4
T = 128                  # grid targets

_CACHE = {}


def _build_nc(n=N, ncores=NCORES, t=T):
    key = (n, ncores, t)
    if key in _CACHE:
        return _CACHE[key]
    jtiles = (n // ncores) // 128
    f32 = mybir.dt.float32
    f16 = mybir.dt.float16
    Exp = mybir.ActivationFunctionType.Exp

    nc = bacc.Bacc("TRN2", target_bir_lowering=False, debug=False)
    # one [128, 128] f32 input (512B rows -> line-rate DMA):
    #   cols 2m/2m+1 (m = c*jtiles+jt): scale' = 2*a*dg, bias' = 2*a*lo-a^2
    #   col 64+m: (Y[j,c], 1.0) packed as two fp16 -> matmul lhsT via bitcast
    sbst_d = nc.dram_tensor("sbst", [128, 128], f32, kind="ExternalInput")
    out_d = nc.dram_tensor("out", [2, C * t], f32, kind="ExternalOutput")

    sbst = nc.alloc_sbuf_tensor("sbst_sb", [128, 128], f32)
    ramp = nc.alloc_sbuf_tensor("ramp_sb", [128, t], f32)
    args = nc.alloc_sbuf_tensor("args_sb", [128, jtiles * C * t], f32)
    gbuf = nc.alloc_sbuf_tensor("g_sb", [128, jtiles * C * t], f16)
    outsb = nc.alloc_sbuf_tensor("out_sb", [2, C * t], f32)
    acc = nc.alloc_psum_tensor("acc_ps", [128, 2048], f32)

    s_in = nc.alloc_semaphore("s_in")      # input DMA chunks (+16 each)
    s_ramp = nc.alloc_semaphore("s_ramp")  # iota done
    s_argv = nc.alloc_semaphore("s_argv")  # DVE arg ops done
    s_argg = nc.alloc_semaphore("s_argg")  # GpSimd arg ops done
    s_g = nc.alloc_semaphore("s_g")        # exp tiles done
    s_mm = nc.alloc_semaphore("s_mm")      # per-channel accumulation closed
    s_ev = nc.alloc_semaphore("s_ev")      # evacuation copies done
    s_out = nc.alloc_semaphore("s_out")    # output DMA done

    # which engine computes arg (jt, c): alternate DVE / GpSimd
    def arg_eng(jt, c):
        return nc.gpsimd if c == 2 else nc.vector

    # cumulative arg-op counts per engine after each jt batch
    nv = [0] * jtiles
    ng = [0] * jtiles
    v = g = 0
    for jt in range(jtiles):
        for c in range(C):
            if arg_eng(jt, c) is nc.vector:
                v += 1
            else:
                g += 1
        nv[jt], ng[jt] = v, g

    aslc = lambda jt, c: args.ap()[:, (jt * C + c) * t : (jt * C + c + 1) * t]
    gslc = lambda jt, c: gbuf.ap()[:, (jt * C + c) * t : (jt * C + c + 1) * t]

    # --- sync: input DMA (descriptors spread across all 16 hw queues on
    # their own; one dma_start means one completion-signal latency), then
    # the final output DMA (walrus's NEFF epilogue drains the queues, so
    # no completion wait is needed) ---
    nc.sync.dma_start(sbst.ap(), sbst_d.ap()).then_inc(s_in, 16)
    nc.sync.wait_ge(s_ev, C)
    nc.sync.dma_start(out_d.ap(), outsb.ap()).then_inc(s_out, 16)

    # --- scalar: input chunk 1, exp-table warm, the 4 exp ACTs, evac c1 ---
    warm = nc.alloc_sbuf_tensor("warm_sb", [128, 1], f32)
    nc.scalar.activation(warm.ap(), nc.const_aps.scalar_like(0.0, warm.ap()), Exp)
    for jt in range(jtiles):
        nc.scalar.wait_ge(s_argv, nv[jt])
        nc.scalar.wait_ge(s_argg, ng[jt])
        nc.scalar.activation(
            gbuf.ap()[:, jt * C * t : (jt + 1) * C * t],
            args.ap()[:, jt * C * t : (jt + 1) * C * t],
            Exp,
        ).then_inc(s_g)
    nc.scalar.wait_ge(s_mm, 2)
    nc.scalar.copy(
        outsb.ap()[:, t : 2 * t], acc.ap()[32 : 32 + 2, 512 : 512 + t]
    ).then_inc(s_ev)

    # --- vector: 6 arg ops, evac c0 and c2 ---
    nc.vector.wait_ge(s_in, 16)
    nc.vector.wait_ge(s_ramp, 1)
    for jt in range(jtiles):
        for c in range(C):
            if arg_eng(jt, c) is nc.vector:
                k = 2 * (c * jtiles + jt)
                nc.vector.tensor_scalar(
                    aslc(jt, c),
                    ramp.ap(),
                    sbst.ap()[:, k : k + 1],
                    sbst.ap()[:, k + 1 : k + 2],
                    mybir.AluOpType.mult,
                    mybir.AluOpType.add,
                ).then_inc(s_argv)
    nc.vector.wait_ge(s_mm, 1)
    nc.vector.tensor_copy(
        outsb.ap()[:, 0:t], acc.ap()[0:2, 0:t]
    ).then_inc(s_ev)
    nc.vector.wait_ge(s_mm, 3)
    nc.vector.tensor_copy(
        outsb.ap()[:, 2 * t : 3 * t], acc.ap()[64 : 64 + 2, 1024 : 1024 + t]
    ).then_inc(s_ev)

    # --- gpsimd: input chunk 2, iota ramp, 6 arg ops ---
    nc.gpsimd.iota(
        ramp.ap(), [[1, t]], channel_multiplier=0,
        allow_small_or_imprecise_dtypes=True,
    ).then_inc(s_ramp)
    nc.gpsimd.wait_ge(s_in, 16)
    for jt in range(jtiles):
        for c in range(C):
            if arg_eng(jt, c) is nc.gpsimd:
                k = 2 * (c * jtiles + jt)
                nc.gpsimd.tensor_scalar(
                    aslc(jt, c),
                    ramp.ap(),
                    sbst.ap()[:, k : k + 1],
                    sbst.ap()[:, k + 1 : k + 2],
                    mybir.AluOpType.mult,
                    mybir.AluOpType.add,
                ).then_inc(s_argg)

    # --- tensor: 12 fp16 matmuls, 3 channels on distinct col-groups ---
    nc.tensor.wait_ge(s_in, 16)
    for jt in range(jtiles):
        nc.tensor.wait_ge(s_g, jt + 1)
        for c in range(C):
            m = c * jtiles + jt
            mm = nc.tensor.matmul(
                acc.ap()[32 * c : 32 * c + 2, c * 512 : c * 512 + t],
                lhsT=sbst.ap()[:, 64 + m : 64 + m + 1].bitcast(f16),
                rhs=gslc(jt, c),
                start=(jt == 0),
                stop=(jt == jtiles - 1),
                tile_position=(0, 32 * c),
            )
            if jt == jtiles - 1:
                mm.then_inc(s_mm)

    nc.compile()
    _CACHE[key] = nc
    return nc


def _prep_inputs(x, train_X, Y, W, h, n=N, ncores=NCORES, t=T):
    """Host-side prep: projections, grid, per-core scale/bias maps."""
    jsh = n // ncores
    jtiles = jsh // 128
    x64 = np.asarray(x, np.float64)
    t64 = np.asarray(train_X, np.float64)
    W64 = np.asarray(W, np.float64)
    hv = float(np.asarray(h).reshape(-1)[0])
    s = 1.0 / (np.sqrt(2.0) * hv)
    b = (x64 @ W64.T) * s          # queries   [n, C]
    a = (t64 @ W64.T) * s          # training  [n, C]
    a32 = a.astype(np.float32)
    b32 = b.astype(np.float32)

    # uniform grid over the query range with a 2-step margin so every
    # query interpolates from an interior Catmull-Rom stencil
    minv = float(b32.min())
    maxv = float(b32.max())
    dg = (maxv - minv) / (t - 5) if maxv > minv else 1.0
    lo = minv - 2.0 * dg

    Yf = np.asarray(Y, np.float64).astype(np.float32)
    one16 = np.uint32(np.float16(1.0).view(np.uint16)) << np.uint32(16)

    in_maps = []
    for r in range(ncores):
        j0 = r * jsh
        m = np.zeros((128, 128), np.float32)
        mu = m.view(np.uint32)
        for c in range(C):
            for jt in range(jtiles):
                kk = c * jtiles + jt
                aj = a32[j0 + jt * 128 : j0 + (jt + 1) * 128, c].astype(np.float64)
                m[:, 2 * kk] = (2.0 * aj * dg).astype(np.float32)
                m[:, 2 * kk + 1] = (2.0 * aj * lo - aj * aj).astype(np.float32)
                y16 = Yf[j0 + jt * 128 : j0 + (jt + 1) * 128, c].astype(np.float16)
                mu[:, 64 + kk] = y16.view(np.uint16).astype(np.uint32) | one16
        in_maps.append({"sbst": m})
    return in_maps, a32, b32, lo, dg


def _interp_catmull_rom(f, lo, dg, xq, t=T):
    """Cubic Catmull-Rom interpolation of f (uniform grid) at xq."""
    u = (np.asarray(xq, np.float64) - lo) / dg
    i = np.clip(np.floor(u).astype(np.int64), 1, t - 3)
    u = u - i
    fm1, f0, f1, f2 = f[i - 1], f[i], f[i + 1], f[i + 2]
    return 0.5 * (
        2.0 * f0
        + (f1 - fm1) * u
        + (2.0 * fm1 - 5.0 * f0 + 4.0 * f1 - f2) * u * u
        + (3.0 * f0 - fm1 - 3.0 * f1 + f2) * u * u * u
    )


def _combine(results, Y, a32, b32, lo, dg, n=N, t=T):
    """Sum per-core partials, damp, interpolate, self-subtract, divide."""
    num = np.zeros((C, t), np.float64)
    den = np.zeros((C, t), np.float64)
    for res in results:
        o = np.asarray(res["out"], np.float64)  # [2, C*t]
        num += o[0].reshape(C, t)
        den += o[1].reshape(C, t)
    grid = lo + dg * np.arange(t, dtype=np.float64)
    damp = np.exp(-(grid**2))
    num *= damp
    den *= damp

    Yf = np.asarray(Y, np.float64)
    out = np.empty((n, C), np.float64)
    for c in range(C):
        ni = _interp_catmull_rom(num[c], lo, dg, b32[:, c], t)
        di = _interp_catmull_rom(den[c], lo, dg, b32[:, c], t)
        # leave-one-out: remove the j == i term exp(-(b_i - a_i)^2)
        kii = np.exp(
            -((b32[:, c].astype(np.float64) - a32[:, c].astype(np.float64)) ** 2)
        )
        out[:, c] = (ni - kii * Yf[:, c]) / (di - kii)
    return out.astype(np.float32)


def kernel(x, train_X, Y, W, h):
    nc = _build_nc()
    in_maps, a32, b32, lo, dg = _prep_inputs(x, train_X, Y, W, h)
    res = run_bass_kernel_spmd(nc, in_maps, core_ids=list(range(NCORES)))
    return _combine(res.results, Y, a32, b32, lo, dg)


# revision 15
# speedup vs baseline: 1.8445x; 1.0845x over previous
"""Trainium2 Bass kernel for leave-one-out Nadaraya-Watson regression
(nn_Net_41420664602632, retrieval_knn).

Math
----
reference:
    Fx = x @ W.T ; Ft = train_X @ W.T          [N, 3]
    K[j,i,c] = exp(-((Ft[j,c]-Fx[i,c])/h)^2/2), K[i,i,c] = 0
    out[i,c] = sum_j K[j,i,c]*Y[j,c] / sum_j K[j,i,c]

With a = Ft/(sqrt(2)*h), b = Fx/(sqrt(2)*h) this is, per channel, a 1-D
Gaussian kernel regression: out[i] = numt(b_i)/dent(b_i) with
    numt(t) = sum_j Y_j exp(-(t-a_j)^2),  dent(t) = sum_j exp(-(t-a_j)^2)
numt/dent are Gaussian-smoothed fields with fixed width 1 in t-space
(the 1/(sqrt(2)h) scaling normalizes the bandwidth away), so instead of
evaluating them at all N=4096 query points (O(N^2) pairwise exps), the
device evaluates them on a uniform T=128-point grid covering the query
range (O(N*T)), and the host Catmull-Rom-interpolates at the 4096 query
positions (grid step ~0.03 of data range << kernel width => interp error
~3e-4 relative, far inside the 2e-2 gate; validated vs reference).

Device program (per core, j-shard of 512 training points)
---------------------------------------------------------
Hand-scheduled Bass (no TileContext): per-engine instruction queues with
manual semaphores. The TileContext scheduler adds ~0.3-0.7us of
semaphore bookkeeping around every instruction plus a ~5us exit sweep
that resets every allocated semaphore on every engine; with only ~50
real instructions this overhead dominated, so the program is wired by
hand (no buffer reuse -> no WAR hazards, 7 semaphores total).

The grid is an fp32 iota 0..T-1 (no DMA), and the affine grid transform
is folded into per-partition scalars computed on the host:
    arg[j,t] = (2 a_j dg) * t + (2 a_j lo - a_j^2)
Per j-tile: 3 tensor_scalar ops (split across DVE and GpSimd) build
arg[128, 3T], one ScalarE ACT takes exp of the whole tile into fp16,
and 3 single-pass fp16 PE matmuls [Y_j, 1]^T @ E accumulate num/den per
channel into distinct PSUM banks / PE col-groups.
Host sums the 8 cores' [2, 3T] partials, multiplies by exp(-g^2),
interpolates at b, subtracts the j==i self term, and divides.
"""

import numpy as np

import concourse.bass as bass
from concourse import bacc, mybir
from concourse.bass_utils import run_bass_kernel_spmd

N = 4096       # training/query points
C = 3          # projected channels (fc1 out_features)
NCORES = 8
JSH = N // NCORES        # 512: j-shard per core
JTILES = JSH // 128      # 4
T = 64                   # grid targets

_CACHE = {}


def _build_nc(n=N, ncores=NCORES, t=T):
    key = (n, ncores, t)
    if key in _CACHE:
        return _CACHE[key]
    jtiles = (n // ncores) // 128
    f32 = mybir.dt.float32
    f16 = mybir.dt.float16
    Exp = mybir.ActivationFunctionType.Exp

    nc = bacc.Bacc("TRN2", target_bir_lowering=False, debug=False)
    # one [128, 128] f32 input (512B rows -> line-rate DMA):
    #   cols 2m/2m+1 (m = c*jtiles+jt): scale' = 2*a*dg, bias' = 2*a*lo-a^2
    #   col 64+m: (Y[j,c], 1.0) packed as two fp16 -> matmul lhsT via bitcast
    sbst_d = nc.dram_tensor("sbst", [128, 128], f32, kind="ExternalInput")
    out_d = nc.dram_tensor("out", [2, C * t], f32, kind="ExternalOutput")

    sbst = nc.alloc_sbuf_tensor("sbst_sb", [128, 128], f32)
    ramp = nc.alloc_sbuf_tensor("ramp_sb", [128, t], f32)
    args = nc.alloc_sbuf_tensor("args_sb", [128, jtiles * C * t], f32)
    gbuf = nc.alloc_sbuf_tensor("g_sb", [128, jtiles * C * t], f16)
    outsb = nc.alloc_sbuf_tensor("out_sb", [2, C * t], f32)
    acc = nc.alloc_psum_tensor("acc_ps", [128, 2048], f32)

    s_in = nc.alloc_semaphore("s_in")      # input DMA chunks (+16 each)
    s_ramp = nc.alloc_semaphore("s_ramp")  # iota done
    s_argv = nc.alloc_semaphore("s_argv")  # DVE arg ops done
    s_argg = nc.alloc_semaphore("s_argg")  # GpSimd arg ops done
    s_g = nc.alloc_semaphore("s_g")        # exp tiles done
    s_mm = nc.alloc_semaphore("s_mm")      # per-channel accumulation closed
    s_ev = nc.alloc_semaphore("s_ev")      # evacuation copies done
    s_out = nc.alloc_semaphore("s_out")    # output DMA done

    # which engine computes arg (jt, c): alternate DVE / GpSimd
    def arg_eng(jt, c):
        return nc.gpsimd if c == 2 else nc.vector

    # cumulative arg-op counts per engine after each jt batch
    nv = [0] * jtiles
    ng = [0] * jtiles
    v = g = 0
    for jt in range(jtiles):
        for c in range(C):
            if arg_eng(jt, c) is nc.vector:
                v += 1
            else:
                g += 1
        nv[jt], ng[jt] = v, g

    aslc = lambda jt, c: args.ap()[:, (jt * C + c) * t : (jt * C + c + 1) * t]
    gslc = lambda jt, c: gbuf.ap()[:, (jt * C + c) * t : (jt * C + c + 1) * t]

    # --- sync: input DMA (descriptors spread across all 16 hw queues on
    # their own; one dma_start means one completion-signal latency), then
    # the final output DMA (walrus's NEFF epilogue drains the queues, so
    # no completion wait is needed) ---
    nc.sync.dma_start(sbst.ap()[:, 0:64], sbst_d.ap()[:, 0:64]).then_inc(s_in, 16)
    nc.sync.dma_start(sbst.ap()[:, 64:128], sbst_d.ap()[:, 64:128]).then_inc(
        s_in, 16
    )

    # --- scalar: input chunk 1, exp-table warm, the 4 exp ACTs, evac c1 ---
    warm = nc.alloc_sbuf_tensor("warm_sb", [128, 1], f32)
    nc.scalar.activation(warm.ap(), nc.const_aps.scalar_like(0.0, warm.ap()), Exp)
    for jt in range(jtiles):
        nc.scalar.wait_ge(s_argv, nv[jt])
        nc.scalar.wait_ge(s_argg, ng[jt])
        nc.scalar.activation(
            gbuf.ap()[:, jt * C * t : (jt + 1) * C * t],
            args.ap()[:, jt * C * t : (jt + 1) * C * t],
            Exp,
        ).then_inc(s_g)
    nc.scalar.wait_ge(s_mm, 2)
    nc.scalar.copy(
        outsb.ap()[:, t : 2 * t], acc.ap()[32 : 32 + 2, 512 : 512 + t]
    ).then_inc(s_ev)

    # --- vector: 6 arg ops, evac c0 and c2 ---
    nc.vector.wait_ge(s_in, 16)
    nc.vector.wait_ge(s_ramp, 1)
    for jt in range(jtiles):
        for c in range(C):
            if arg_eng(jt, c) is nc.vector:
                k = 2 * (c * jtiles + jt)
                nc.vector.tensor_scalar(
                    aslc(jt, c),
                    ramp.ap(),
                    sbst.ap()[:, k : k + 1],
                    sbst.ap()[:, k + 1 : k + 2],
                    mybir.AluOpType.mult,
                    mybir.AluOpType.add,
                ).then_inc(s_argv)
    nc.vector.wait_ge(s_mm, 1)
    nc.vector.tensor_copy(
        outsb.ap()[:, 0:t], acc.ap()[0:2, 0:t]
    ).then_inc(s_ev)
    nc.vector.wait_ge(s_mm, 3)
    nc.vector.tensor_copy(
        outsb.ap()[:, 2 * t : 3 * t], acc.ap()[64 : 64 + 2, 1024 : 1024 + t]
    ).then_inc(s_ev)

    # --- gpsimd: input chunk 2, iota ramp, 6 arg ops ---
    nc.gpsimd.iota(
        ramp.ap(), [[1, t]], channel_multiplier=0,
        allow_small_or_imprecise_dtypes=True,
    ).then_inc(s_ramp)
    nc.gpsimd.wait_ge(s_in, 16)
    for jt in range(jtiles):
        for c in range(C):
            if arg_eng(jt, c) is nc.gpsimd:
                k = 2 * (c * jtiles + jt)
                nc.gpsimd.tensor_scalar(
                    aslc(jt, c),
                    ramp.ap(),
                    sbst.ap()[:, k : k + 1],
                    sbst.ap()[:, k + 1 : k + 2],
                    mybir.AluOpType.mult,
                    mybir.AluOpType.add,
                ).then_inc(s_argg)
    nc.gpsimd.wait_ge(s_ev, C)
    nc.gpsimd.dma_start(out_d.ap(), outsb.ap()).then_inc(s_out, 16)

    # --- tensor: 12 fp16 matmuls, 3 channels on distinct col-groups ---
    nc.tensor.wait_ge(s_in, 32)
    for jt in range(jtiles):
        nc.tensor.wait_ge(s_g, jt + 1)
        for c in range(C):
            m = c * jtiles + jt
            mm = nc.tensor.matmul(
                acc.ap()[32 * c : 32 * c + 2, c * 512 : c * 512 + t],
                lhsT=sbst.ap()[:, 64 + m : 64 + m + 1].bitcast(f16),
                rhs=gslc(jt, c),
                start=(jt == 0),
                stop=(jt == jtiles - 1),
                tile_position=(0, 32 * c),
            )
            if jt == jtiles - 1:
                mm.then_inc(s_mm)

    nc.compile()
    _CACHE[key] = nc
    return nc


def _prep_inputs(x, train_X, Y, W, h, n=N, ncores=NCORES, t=T):
    """Host-side prep: projections, grid, per-core scale/bias maps."""
    jsh = n // ncores
    jtiles = jsh // 128
    x64 = np.asarray(x, np.float64)
    t64 = np.asarray(train_X, np.float64)
    W64 = np.asarray(W, np.float64)
    hv = float(np.asarray(h).reshape(-1)[0])
    s = 1.0 / (np.sqrt(2.0) * hv)
    b = (x64 @ W64.T) * s          # queries   [n, C]
    a = (t64 @ W64.T) * s          # training  [n, C]
    a32 = a.astype(np.float32)
    b32 = b.astype(np.float32)

    # uniform grid over the query range with a 2-step margin so every
    # query interpolates from an interior Catmull-Rom stencil
    minv = float(b32.min())
    maxv = float(b32.max())
    dg = (maxv - minv) / (t - 5) if maxv > minv else 1.0
    lo = minv - 2.0 * dg

    Yf = np.asarray(Y, np.float64).astype(np.float32)
    one16 = np.uint32(np.float16(1.0).view(np.uint16)) << np.uint32(16)

    in_maps = []
    for r in range(ncores):
        j0 = r * jsh
        m = np.zeros((128, 128), np.float32)
        mu = m.view(np.uint32)
        for c in range(C):
            for jt in range(jtiles):
                kk = c * jtiles + jt
                aj = a32[j0 + jt * 128 : j0 + (jt + 1) * 128, c].astype(np.float64)
                m[:, 2 * kk] = (2.0 * aj * dg).astype(np.float32)
                m[:, 2 * kk + 1] = (2.0 * aj * lo - aj * aj).astype(np.float32)
                y16 = Yf[j0 + jt * 128 : j0 + (jt + 1) * 128, c].astype(np.float16)
                mu[:, 64 + kk] = y16.view(np.uint16).astype(np.uint32) | one16
        in_maps.append({"sbst": m})
    return in_maps, a32, b32, lo, dg


def _interp_catmull_rom(f, lo, dg, xq, t=T):
    """Cubic Catmull-Rom interpolation of f (uniform grid) at xq."""
    u = (np.asarray(xq, np.float64) - lo) / dg
    i = np.clip(np.floor(u).astype(np.int64), 1, t - 3)
    u = u - i
    fm1, f0, f1, f2 = f[i - 1], f[i], f[i + 1], f[i + 2]
    return 0.5 * (
        2.0 * f0
        + (f1 - fm1) * u
        + (2.0 * fm1 - 5.0 * f0 + 4.0 * f1 - f2) * u * u
        + (3.0 * f0 - fm1 - 3.0 * f1 + f2) * u * u * u
    )


def _combine(results, Y, a32, b32, lo, dg, n=N, t=T):
    """Sum per-core partials, damp, interpolate, self-subtract, divide."""
    num = np.zeros((C, t), np.float64)
    den = np.zeros((C, t), np.float64)
    for res in results:
        o = np.asarray(res["out"], np.float64)  # [2, C*t]
        num += o[0].reshape(C, t)
        den += o[1].reshape(C, t)
    grid = lo + dg * np.arange(t, dtype=np.float64)
    damp = np.exp(-(grid**2))
    num *= damp
    den *= damp

    Yf = np.asarray(Y, np.float64)
    out = np.empty((n, C), np.float64)
    for c in range(C):
        ni = _interp_catmull_rom(num[c], lo, dg, b32[:, c], t)
        di = _interp_catmull_rom(den[c], lo, dg, b32[:, c], t)
        # leave-one-out: remove the j == i term exp(-(b_i - a_i)^2)
        kii = np.exp(
            -((b32[:, c].astype(np.float64) - a32[:, c].astype(np.float64)) ** 2)
        )
        out[:, c] = (ni - kii * Yf[:, c]) / (di - kii)
    return out.astype(np.float32)


def kernel(x, train_X, Y, W, h):
    nc = _build_nc()
    in_maps, a32, b32, lo, dg = _prep_inputs(x, train_X, Y, W, h)
    res = run_bass_kernel_spmd(nc, in_maps, core_ids=list(range(NCORES)))
    return _combine(res.results, Y, a32, b32, lo, dg)
